# revision 33
# baseline (speedup 1.0000x reference)
"""DetectorLoss on 8 Trainium2 NeuronCores (Bass/Tile).

Strategy (data-parallel over batch, 4 images per core):
  * The loss only touches pred_cls / pred_delta_box at ~P sparse "positive"
    locations, so those tensors are never streamed — dma_gather fetches the
    256B blocks containing the needed elements; one-hot masks (built on host
    from the integer indices) extract them with a multiply+reduce.
  * pred_obj is streamed in full (1.23MB/core) for the SmoothL1 background
    sum; positive-cell corrections are applied from the sparse gather.
  * All index arithmetic (offsets, duplicate-winner mask, per-image counts)
    is integer work done on host while sharding; all float math runs on
    device. Host combines per-core partial sums (weighted means).

Background sum identity used on device (per element x):
  sl1(x) = min(|x|,1)*(|x| - 0.5*min(|x|,1))
  sum sl1 = T1 - 0.5*T2,  T1 = sum min(|x|,1)*|x|,  T2 = sum min(x^2, 1)
"""
import numpy as np

B, A, C, H, W = 32, 3, 20, 160, 160
HW = H * W
M = 8            # cores
Bm = B // M      # images per core
SZ_OBJ = Bm * A * HW          # 307200
SZ_DBOX = Bm * A * 4 * HW     # 1228800
SZ_CLS = Bm * A * C * HW      # 6144000
NTOT = SZ_OBJ + SZ_DBOX + SZ_CLS
BASE_DBOX = SZ_OBJ
BASE_CLS = SZ_OBJ + SZ_DBOX
NCHUNK = 4
F = (SZ_OBJ // 128) // NCHUNK  # 600
NF = 11                        # host-data planes
EPS = 1e-7
ES = 64                        # gather element size (f32) = 256B
NBLK0 = (SZ_OBJ + SZ_DBOX) // ES   # 24000 blocks in obj+dbox window
ESC = 192                          # cls gather element size (768B blocks)
NBLKC = SZ_CLS // ESC              # 32000 blocks — fits int16 in ONE window

_NC_CACHE = {}


def _build_nc(K):
    import concourse.bass as bass
    import concourse.bacc as bacc
    import concourse.tile as tile
    from concourse import mybir

    f32 = mybir.dt.float32
    i16 = mybir.dt.int16
    op = mybir.AluOpType
    act = mybir.ActivationFunctionType
    LP = 128 * K

    nc = bacc.Bacc("TRN2", target_bir_lowering=False, debug=False)
    preds_p = nc.dram_tensor("preds", [NTOT, 1], f32, kind="ExternalInput")
    idx0_p = nc.dram_tensor("idx0", [128, 2 * LP // 16], i16, kind="ExternalInput")
    idxc_p = nc.dram_tensor("idxc", [128, LP // 16], i16, kind="ExternalInput")
    mask0_p = nc.dram_tensor("mask0", [128, K, ES], f32, kind="ExternalInput")
    maskd_p = nc.dram_tensor("maskd", [128, 4 * K, ES], f32, kind="ExternalInput")
    maskc_p = nc.dram_tensor("maskc", [128, K, ESC], f32, kind="ExternalInput")
    hd_p = nc.dram_tensor("hd", [128, NF * K], f32, kind="ExternalInput")
    NCOLS = 2 * NCHUNK + 3
    out_p = nc.dram_tensor("partials", [128, NCOLS], f32, kind="ExternalOutput")

    with tile.TileContext(nc) as tc, \
         tc.tile_pool(name="io", bufs=1) as io, \
         tc.tile_pool(name="wk", bufs=1) as wk, \
         tc.tile_pool(name="stream", bufs=5) as st:
        idx0 = io.tile([128, 2 * LP // 16], i16)
        idxc = io.tile([128, LP // 16], i16)
        mask0 = io.tile([128, K, ES], f32)
        maskd = io.tile([128, 4 * K, ES], f32)
        maskc = io.tile([128, K, ESC], f32)
        hd = io.tile([128, NF * K], f32)
        g0 = io.tile([128, 2 * K, ES], f32)
        gc = io.tile([128, K, ESC], f32)
        partials = io.tile([128, NCOLS], f32)
        # activation-table warmers: demand {ln, exp} first so the table pass
        # loads a covering set once at t=0 instead of mid-kernel.
        warm = wk.tile([128, 1], f32, name="warm", tag="warm")
        nc.vector.memset(warm[:, :], 1.0)
        warm2 = wk.tile([128, 1], f32, name="warm2", tag="warm2")
        nc.scalar.activation(out=warm2[:, :], in_=warm[:, :], func=act.Ln)
        warm3 = wk.tile([128, 1], f32, name="warm3", tag="warm3")
        nc.scalar.activation(out=warm3[:, :], in_=warm[:, :], func=act.Exp)
        # tiny DMAs first (gathers depend on idx tiles); big masks go on the
        # idle tensor-engine HWDGE queue so they don't delay the gathers.
        nc.sync.dma_start(out=idx0[:, :], in_=idx0_p[:, :])
        nc.sync.dma_start(out=idxc[:, :], in_=idxc_p[:, :])
        nc.sync.dma_start(out=hd[:, :], in_=hd_p[:, :])
        nc.scalar.dma_start(out=mask0[:, :, :], in_=mask0_p[:, :, :])
        nc.scalar.dma_start(out=maskd[:, :, :], in_=maskd_p[:, :, :])
        nc.scalar.dma_start(out=maskc[:, :, :], in_=maskc_p[:, :, :])

        # ---- streamed background pass over pred_obj, entirely on ACT:
        # sum sl1(x) = 0.5*(sum x^2 - sum relu(|x|-1)^2), so per chunk we
        # accumulate QS = sum Square(x) and RD = sum Square(Relu(|x|-1)).
        # Keeps the (critical) in-order DVE queue completely free.
        negone = wk.tile([128, 1], f32, name="negone", tag="negone")
        nc.vector.memset(negone[:, :], -1.0)
        for c in range(NCHUNK):
            x = st.tile([128, F], f32)
            chunk = bass.AP(tensor=preds_p[:, :].tensor, offset=c * 128 * F,
                            ap=[[F, 128], [1, F]])
            nc.sync.dma_start(out=x[:, :], in_=chunk)
            q = st.tile([128, F], f32)
            nc.scalar.activation(out=q[:, :], in_=x[:, :], func=act.Square,
                                 accum_out=partials[:, c:c + 1])
            a = st.tile([128, F], f32)
            nc.scalar.activation(out=a[:, :], in_=x[:, :], func=act.Abs)
            r = st.tile([128, F], f32)
            nc.scalar.activation(out=r[:, :], in_=a[:, :], func=act.Relu,
                                 bias=negone[:, 0:1])
            r2 = st.tile([128, F], f32)
            nc.scalar.activation(out=r2[:, :], in_=r[:, :], func=act.Square,
                                 accum_out=partials[:, NCHUNK + c:NCHUNK + c + 1])

        # sparse gathers: 256B blocks around each positive element.
        # >512 idxs in one InstDMAGatherAnt crashes the exec unit — chunk it.
        # plane 0: pred_obj elements; plane 1: 16B groups (d0..d3) from the
        # host-transposed pred_delta_box region.
        GMAX = 512
        # pod gathers first: the d extraction gates the long SIoU chain
        w0_ap = bass.AP(tensor=preds_p[:, :].tensor, offset=0,
                        ap=[[ES, NBLK0], [1, ES]])
        for st_ in range(0, 2 * LP, GMAX):
            n = min(GMAX, 2 * LP - st_)
            nc.gpsimd.dma_gather(
                out_ap=g0[:, st_ // 128:(st_ + n) // 128, :], in_ap=w0_ap,
                idxs_ap=idx0[:, st_ // 16:(st_ + n) // 16],
                num_idxs=n, num_idxs_reg=n, elem_size=ES)
        wc_ap = bass.AP(tensor=preds_p[:, :].tensor, offset=BASE_CLS,
                        ap=[[ESC, NBLKC], [1, ESC]])
        for st_ in range(0, LP, GMAX):
            n = min(GMAX, LP - st_)
            nc.gpsimd.dma_gather(
                out_ap=gc[:, st_ // 128:(st_ + n) // 128, :], in_ap=wc_ap,
                idxs_ap=idxc[:, st_ // 16:(st_ + n) // 16],
                num_idxs=n, num_idxs_reg=n, elem_size=ESC)

        # ---- extraction: one-hot mask multiply + reduce over the 64-lane dim
        e0p = wk.tile([128, K, ES], f32)
        nc.vector.tensor_tensor(out=e0p[:, :, :], in0=g0[:, 0:K, :],
                                in1=mask0[:, :, :], op=op.mult)
        ext0 = wk.tile([128, K], f32)
        nc.vector.tensor_reduce(out=ext0[:, :], in_=e0p[:, :, :],
                                axis=mybir.AxisListType.X, op=op.add)
        # d0..d3 from the single d plane: broadcast the gathered rows 4x
        # against 4 one-hot mask sets (rem+k), then reduce.
        gd = g0[:, K:2 * K, :]
        gd_b = bass.AP(tensor=gd.tensor, offset=gd.offset,
                       ap=[gd.ap[0], [0, 4], gd.ap[1], gd.ap[2]])
        edp = wk.tile([128, 4, K, ES], f32)
        nc.vector.tensor_tensor(out=edp[:, :, :, :], in0=gd_b,
                                in1=maskd[:, :, :].rearrange(
                                    "p (four k) e -> p four k e", four=4),
                                op=op.mult)
        dext = wk.tile([128, 4 * K], f32)
        nc.vector.tensor_reduce(
            out=dext[:, :], in_=edp[:, :, :, :],
            axis=mybir.AxisListType.X, op=op.add)
        # cls mask-multiply on GpSimd: keeps the (gather-gated) op off the
        # in-order DVE queue so it can't starve ready DVE work behind it.
        ecp = wk.tile([128, K, ESC], f32)
        nc.gpsimd.tensor_tensor(out=ecp[:, :, :], in0=gc[:, :, :],
                                in1=maskc[:, :, :], op=op.mult)
        pcg = wk.tile([128, K], f32)
        nc.vector.tensor_reduce(out=pcg[:, :], in_=ecp[:, :, :],
                                axis=mybir.AxisListType.X, op=op.add)

        # ---- per-positive math (x/y packed side by side in [128, 2K]) ----
        PR = [128, 2 * K]
        SG = [128, K]
        import itertools
        _cnt = itertools.count()

        def pr():
            n = f"pr{next(_cnt)}"
            return wk.tile(PR, f32, name=n, tag=n)

        def sg():
            n = f"sg{next(_cnt)}"
            return wk.tile(SG, f32, name=n, tag=n)

        po = ext0[:, 0:K]
        d01 = dext[:, 0:2 * K]
        d23 = dext[:, 2 * K:4 * K]
        pxy = hd[:, 0:2 * K]
        anc = hd[:, 2 * K:4 * K]
        gtc = hd[:, 4 * K:6 * K]
        gtwh = hd[:, 6 * K:8 * K]
        cnt_ = hd[:, 8 * K:9 * K]
        win = hd[:, 9 * K:10 * K]
        valid = hd[:, 10 * K:11 * K]

        tt = nc.vector.tensor_tensor
        ts = nc.vector.tensor_scalar
        stt = nc.vector.scalar_tensor_tensor
        A_ = nc.scalar.activation

        def lohalf(t):
            return t[:, 0:K]

        def hihalf(t):
            return t[:, K:2 * K]

        # tanh via exp keeps the ACT table to {exp, ln} (one table set, no
        # mid-kernel LoadActFuncSet): tanh(x) = (e^{2x}-1)/(e^{2x}+1)
        e2 = pr(); A_(out=e2[:], in_=d01, func=act.Exp, scale=2.0)
        e2p = pr(); ts(out=e2p[:], in0=e2[:], scalar1=1.0, scalar2=None, op0=op.add)
        re2 = pr(); nc.vector.reciprocal(out=re2[:], in_=e2p[:])
        e2m = pr(); ts(out=e2m[:], in0=e2[:], scalar1=-1.0, scalar2=None, op0=op.add)
        th = pr(); tt(out=th[:], in0=e2m[:], in1=re2[:], op=op.mult)
        ex = pr(); A_(out=ex[:], in_=d23, func=act.Exp)
        c1 = pr(); tt(out=c1[:], in0=th[:], in1=pxy, op=op.add)
        wh1 = pr(); stt(out=wh1[:], in0=ex[:], scalar=float(W), in1=anc,
                        op0=op.mult, op1=op.mult)
        b1lo = pr(); stt(out=b1lo[:], in0=wh1[:], scalar=-0.5, in1=c1[:],
                         op0=op.mult, op1=op.add)
        b1hi = pr(); stt(out=b1hi[:], in0=wh1[:], scalar=0.5, in1=c1[:],
                         op0=op.mult, op1=op.add)
        b2lo = pr(); stt(out=b2lo[:], in0=gtwh, scalar=-0.5, in1=gtc,
                         op0=op.mult, op1=op.add)
        b2hi = pr(); stt(out=b2hi[:], in0=gtwh, scalar=0.5, in1=gtc,
                         op0=op.mult, op1=op.add)
        mnhi = pr(); tt(out=mnhi[:], in0=b1hi[:], in1=b2hi[:], op=op.min)
        mxlo = pr(); tt(out=mxlo[:], in0=b1lo[:], in1=b2lo[:], op=op.max)
        itax = pr(); tt(out=itax[:], in0=mnhi[:], in1=mxlo[:], op=op.subtract)
        itax2 = pr(); ts(out=itax2[:], in0=itax[:], scalar1=0.0, scalar2=None,
                         op0=op.max)
        inter = sg(); tt(out=inter[:], in0=lohalf(itax2), in1=hihalf(itax2),
                         op=op.mult)
        epsv = wk.tile(PR, f32, tag="epsv")
        nc.vector.memset(epsv[:, 0:K], 0.0)
        nc.vector.memset(epsv[:, K:2 * K], EPS)
        whe1 = pr(); tt(out=whe1[:], in0=b1hi[:], in1=b1lo[:], op=op.subtract)
        whe1b = pr(); tt(out=whe1b[:], in0=whe1[:], in1=epsv[:], op=op.add)
        whe2 = pr(); tt(out=whe2[:], in0=b2hi[:], in1=b2lo[:], op=op.subtract)
        whe2b = pr(); tt(out=whe2b[:], in0=whe2[:], in1=epsv[:], op=op.add)
        area1 = sg(); tt(out=area1[:], in0=lohalf(whe1b), in1=hihalf(whe1b),
                         op=op.mult)
        area2 = sg(); tt(out=area2[:], in0=lohalf(whe2b), in1=hihalf(whe2b),
                         op=op.mult)
        u1 = sg(); tt(out=u1[:], in0=area1[:], in1=area2[:], op=op.add)
        u2 = sg(); tt(out=u2[:], in0=u1[:], in1=inter[:], op=op.subtract)
        u3 = sg(); ts(out=u3[:], in0=u2[:], scalar1=EPS, scalar2=None, op0=op.add)
        ru = sg(); nc.vector.reciprocal(out=ru[:], in_=u3[:])
        iou = sg(); tt(out=iou[:], in0=inter[:], in1=ru[:], op=op.mult)
        cwmax = pr(); tt(out=cwmax[:], in0=b1hi[:], in1=b2hi[:], op=op.max)
        cwmin = pr(); tt(out=cwmin[:], in0=b1lo[:], in1=b2lo[:], op=op.min)
        cw = pr(); tt(out=cw[:], in0=cwmax[:], in1=cwmin[:], op=op.subtract)
        s1 = pr(); tt(out=s1[:], in0=b2lo[:], in1=b2hi[:], op=op.add)
        s2 = pr(); tt(out=s2[:], in0=b1lo[:], in1=b1hi[:], op=op.add)
        sdf = pr(); stt(out=sdf[:], in0=s2[:], scalar=-1.0, in1=s1[:],
                        op0=op.mult, op1=op.add)
        s = pr(); ts(out=s[:], in0=sdf[:], scalar1=0.5, scalar2=None, op0=op.mult)
        # angle_cost = sin(2*arcsin(min|s|/sigma)) = 2*|s_cw*s_ch| / sigma^2
        # (sqrt-free: min*max of |s| halves = |product|)
        sqs = pr(); tt(out=sqs[:], in0=s[:], in1=s[:], op=op.mult)
        sig2 = sg(); tt(out=sig2[:], in0=lohalf(sqs), in1=hihalf(sqs), op=op.add)
        prod = sg(); tt(out=prod[:], in0=lohalf(s), in1=hihalf(s), op=op.mult)
        aprod = sg(); stt(out=aprod[:], in0=prod[:], scalar=-1.0, in1=prod[:],
                          op0=op.mult, op1=op.max)
        rsig2 = sg(); nc.vector.reciprocal(out=rsig2[:], in_=sig2[:])
        angle = sg(); stt(out=angle[:], in0=aprod[:], scalar=2.0, in1=rsig2[:],
                          op0=op.mult, op1=op.mult)
        gamma = sg(); ts(out=gamma[:], in0=angle[:], scalar1=-2.0, scalar2=None,
                         op0=op.add)
        rcw = pr(); nc.vector.reciprocal(out=rcw[:], in_=cw[:])
        srw = pr(); tt(out=srw[:], in0=s[:], in1=rcw[:], op=op.mult)
        rho = pr(); tt(out=rho[:], in0=srw[:], in1=srw[:], op=op.mult)
        # pack dist-cost exps and -omiga into one [4K] tile so a single ACT
        # Exp covers both (last ln->exp set switch disappears)
        wd = pr(); tt(out=wd[:], in0=whe1b[:], in1=whe2b[:], op=op.subtract)
        wda = pr(); stt(out=wda[:], in0=wd[:], scalar=-1.0, in1=wd[:],
                        op0=op.mult, op1=op.max)
        mxw = pr(); tt(out=mxw[:], in0=whe1b[:], in1=whe2b[:], op=op.max)
        rmx = pr(); nc.vector.reciprocal(out=rmx[:], in_=mxw[:])
        grho4 = wk.tile([128, 4 * K], f32, tag="grho4")
        tt(out=grho4[:, 0:K], in0=gamma[:], in1=rho[:, 0:K], op=op.mult)
        tt(out=grho4[:, K:2 * K], in0=gamma[:], in1=rho[:, K:2 * K], op=op.mult)
        stt(out=grho4[:, 2 * K:4 * K], in0=wda[:], scalar=-1.0, in1=rmx[:],
            op0=op.mult, op1=op.mult)
        e4 = wk.tile([128, 4 * K], f32, tag="e4")
        A_(out=e4[:, :], in_=grho4[:, :], func=act.Exp)
        egs = sg(); tt(out=egs[:], in0=e4[:, 0:K], in1=e4[:, K:2 * K], op=op.add)
        dist = sg(); ts(out=dist[:], in0=egs[:], scalar1=-1.0, scalar2=2.0,
                        op0=op.mult, op1=op.add)
        oneo = pr(); ts(out=oneo[:], in0=e4[:, 2 * K:4 * K], scalar1=-1.0,
                        scalar2=1.0, op0=op.mult, op1=op.add)
        sq1 = pr(); tt(out=sq1[:], in0=oneo[:], in1=oneo[:], op=op.mult)
        sh = pr(); tt(out=sh[:], in0=sq1[:], in1=sq1[:], op=op.mult)
        shs = sg(); tt(out=shs[:], in0=lohalf(sh), in1=hihalf(sh), op=op.add)
        ds = sg(); tt(out=ds[:], in0=dist[:], in1=shs[:], op=op.add)
        siou = sg(); stt(out=siou[:], in0=ds[:], scalar=-0.5, in1=iou[:],
                         op0=op.mult, op1=op.add)
        onem = sg(); ts(out=onem[:], in0=siou[:], scalar1=-1.0, scalar2=1.0,
                        op0=op.mult, op1=op.add)
        jk1 = sg(); stt(out=jk1[:], in0=onem[:], scalar=1.0, in1=valid,
                        op0=op.mult, op1=op.mult,
                        accum_out=partials[:, 2 * NCHUNK:2 * NCHUNK + 1])
        lnp = sg(); A_(out=lnp[:], in_=pcg[:, :], func=act.Ln)
        jk2 = sg(); stt(out=jk2[:], in0=lnp[:], scalar=-1.0, in1=valid,
                        op0=op.mult, op1=op.mult,
                        accum_out=partials[:, 2 * NCHUNK + 1:2 * NCHUNK + 2])
        dif = sg(); tt(out=dif[:], in0=po, in1=siou[:], op=op.subtract)
        ad = sg(); stt(out=ad[:], in0=dif[:], scalar=-1.0, in1=dif[:],
                       op0=op.mult, op1=op.max)
        md = sg(); ts(out=md[:], in0=ad[:], scalar1=1.0, scalar2=None, op0=op.min)
        ud = sg(); stt(out=ud[:], in0=md[:], scalar=-0.5, in1=ad[:],
                       op0=op.mult, op1=op.add)
        sd = sg(); tt(out=sd[:], in0=md[:], in1=ud[:], op=op.mult)
        rc = sg(); nc.vector.reciprocal(out=rc[:], in_=cnt_)
        t1 = sg(); stt(out=t1[:], in0=rc[:], scalar=float(0.25 * HW), in1=sd[:],
                       op0=op.mult, op1=op.mult)
        a2 = sg(); stt(out=a2[:], in0=po, scalar=-1.0, in1=po,
                       op0=op.mult, op1=op.max)
        m2 = sg(); ts(out=m2[:], in0=a2[:], scalar1=1.0, scalar2=None, op0=op.min)
        u2t = sg(); stt(out=u2t[:], in0=m2[:], scalar=-0.5, in1=a2[:],
                        op0=op.mult, op1=op.add)
        s2m = sg(); stt(out=s2m[:], in0=m2[:], scalar=0.75, in1=u2t[:],
                        op0=op.mult, op1=op.mult)
        corr = sg(); tt(out=corr[:], in0=t1[:], in1=s2m[:], op=op.subtract)
        jk3 = sg(); stt(out=jk3[:], in0=corr[:], scalar=1.0, in1=win,
                        op0=op.mult, op1=op.mult,
                        accum_out=partials[:, 2 * NCHUNK + 2:2 * NCHUNK + 3])

        nc.sync.dma_start(out=out_p[:, :], in_=partials[:, :])

    return nc


def _get_nc(K, finalized=True):
    key = (K, finalized)
    if key not in _NC_CACHE:
        nc = _build_nc(K)
        if finalized:
            nc.finalize()
        else:
            nc.compile()
        _NC_CACHE[key] = nc
    return _NC_CACHE[key]


def _pack(vals, K, fill, dtype):
    """lane j = i*128 + p  ->  tile[p, i]."""
    out = np.full((K, 128), fill, dtype)
    out.reshape(-1)[:len(vals)] = vals
    return out.T


def _wrap_idx16(vals):
    """dma_gather index layout: idx j -> [j%16, j//16], replicated x8 stripes."""
    n = len(vals)
    t = np.zeros((128, n // 16), np.int16)
    j = np.arange(n)
    for s_ in range(8):
        t[16 * s_ + (j % 16), j // 16] = vals
    return t


def host_prep(pred_obj, pred_delta_box, pred_cls, gt_box, gt_cls,
              p_batch_idx, p_x_idx, p_y_idx, p_anchor_idx, anchors):
    """Shard inputs; all-integer index prep. Returns (in_maps, K, P)."""
    f32 = np.float32
    pred_obj = np.asarray(pred_obj, f32)
    pred_delta_box = np.asarray(pred_delta_box, f32)
    pred_cls = np.asarray(pred_cls, f32)
    gt_box = np.asarray(gt_box, f32)
    gt_cls = np.asarray(gt_cls, np.int64)
    p_b = np.asarray(p_batch_idx, np.int64)
    p_x = np.asarray(p_x_idx, np.int64)
    p_y = np.asarray(p_y_idx, np.int64)
    p_a = np.asarray(p_anchor_idx, np.int64)
    anchors = np.asarray(anchors, f32)
    P = len(p_b)

    n_img = np.bincount(p_b, minlength=B)
    # duplicate (b,y,x,a) cells: last occurrence wins (matches XLA scatter)
    cell = ((p_b * H + p_y) * W + p_x) * A + p_a
    win = np.zeros(P, f32)
    _, ridx = np.unique(cell[::-1], return_index=True)
    win[P - 1 - ridx] = 1.0

    core_of = p_b // Bm
    counts = np.bincount(core_of, minlength=M)
    Pmax = int(counts.max())
    K = max(1, -(-Pmax // 128))
    LP = 128 * K

    in_maps = []
    for m in range(M):
        sel = core_of == m
        bl = p_b[sel] - m * Bm
        xj, yj, aj, cj = p_x[sel], p_y[sel], p_a[sel], gt_cls[sel]
        base = bl * A + aj
        sp = yj * W + xj
        off_obj = base * HW + sp
        # dbox region is host-transposed to [Bm,A,H,W,4]: 16B group per cell
        off_d = BASE_DBOX + (base * HW + sp) * 4
        off_cls = BASE_CLS + (base * C + cj) * HW + sp

        # G0: plane 0 = po elements, plane 1 = d 16B groups
        g0_packed = [_pack(off_obj, K, 0, np.int64), _pack(off_d, K, 0, np.int64)]
        idx0_vals = np.concatenate([(o.T.reshape(-1) >> 6) for o in g0_packed])
        idx0 = _wrap_idx16(idx0_vals.astype(np.int16))
        pg, ig = np.meshgrid(np.arange(128), np.arange(K), indexing="ij")
        mask0 = np.zeros((128, K, ES), f32)
        mask0[pg, ig, g0_packed[0] & 63] = 1.0
        maskd = np.zeros((128, 4 * K, ES), f32)
        for k_ in range(4):
            maskd[pg, k_ * K + ig, (g0_packed[1] & 63) + k_] = 1.0

        # cls: one window of 768B blocks covers the whole region
        oc_packed = _pack(off_cls, K, BASE_CLS, np.int64)          # [128,K]
        rel = oc_packed - BASE_CLS
        blk = rel // ESC
        rem = rel % ESC
        idxc = _wrap_idx16((blk.T.reshape(-1)).astype(np.int16))
        # pads (rel=0) extract preds[BASE_CLS] > 0, keeping Ln finite
        maskc = np.zeros((128, K, ESC), f32)
        maskc[pg, ig, rem] = 1.0

        gtb = gt_box[sel]
        ancg = anchors[aj]
        hd_planes = [
            _pack(xj.astype(f32), K, 0.0, f32),
            _pack(yj.astype(f32), K, 0.0, f32),
            _pack(ancg[:, 0], K, 0.1, f32),
            _pack(ancg[:, 1], K, 0.1, f32),
            _pack(gtb[:, 0], K, 0.5, f32),
            _pack(gtb[:, 1], K, 0.5, f32),
            _pack(gtb[:, 2], K, 0.5, f32),
            _pack(gtb[:, 3], K, 0.5, f32),
            _pack(n_img[p_b[sel]].astype(f32), K, 1.0, f32),
            _pack(win[sel], K, 0.0, f32),
            _pack(np.ones(int(sel.sum()), f32), K, 0.0, f32),
        ]
        hd = np.concatenate(hd_planes, axis=1)  # [128, 11K]

        preds = np.empty(NTOT, f32)
        preds[:SZ_OBJ] = pred_obj[m * Bm:(m + 1) * Bm].reshape(-1)
        preds[BASE_DBOX:BASE_CLS] = \
            pred_delta_box[m * Bm:(m + 1) * Bm].transpose(0, 1, 3, 4, 2).reshape(-1)
        preds[BASE_CLS:] = pred_cls[m * Bm:(m + 1) * Bm].reshape(-1)

        in_maps.append({
            "preds": preds.reshape(NTOT, 1),
            "idx0": np.ascontiguousarray(idx0),
            "idxc": np.ascontiguousarray(idxc),
            "mask0": mask0,
            "maskd": maskd,
            "maskc": maskc,
            "hd": np.ascontiguousarray(hd),
        })
    return in_maps, K, P


def combine(partials_list, P):
    """Host reduction of per-core [128, 2*NCHUNK+3] partial sums.

    cols [0:N) = sum x^2, [N:2N) = sum relu(|x|-1)^2;
    background sum sl1 = 0.5*(QS - RD).
    """
    tot_QS = tot_RD = tot_iou = tot_cls = tot_corr = 0.0
    for pt in partials_list:
        pt = np.asarray(pt, np.float64)
        tot_QS += pt[:, :NCHUNK].sum()
        tot_RD += pt[:, NCHUNK:2 * NCHUNK].sum()
        tot_iou += pt[:, 2 * NCHUNK].sum()
        tot_cls += pt[:, 2 * NCHUNK + 1].sum()
        tot_corr += pt[:, 2 * NCHUNK + 2].sum()
    iou_loss = tot_iou / P
    cls_loss = tot_cls / P
    obj_loss = (0.375 * (tot_QS - tot_RD) + tot_corr) / (B * A * H * W)
    tot_loss = iou_loss + 4 * obj_loss + 2 * cls_loss
    return (np.float32(iou_loss), np.float32(obj_loss),
            np.float32(cls_loss), np.float32(tot_loss))


def kernel(pred_obj, pred_delta_box, pred_cls, gt_box, gt_cls,
           p_batch_idx, p_x_idx, p_y_idx, p_anchor_idx, anchors):
    from concourse.bass_utils import run_bass_kernel_spmd
    in_maps, K, P = host_prep(pred_obj, pred_delta_box, pred_cls, gt_box,
                              gt_cls, p_batch_idx, p_x_idx, p_y_idx,
                              p_anchor_idx, anchors)
    nc = _get_nc(K)
    res = run_bass_kernel_spmd(nc, in_maps, list(range(M))).results
    return combine([r["partials"] for r in res], P)
